# revision 7
# baseline (speedup 1.0000x reference)
"""Trainium2 Bass kernel for nn_NoiseFilter.

Math (derived from the reference, validated to ~6e-7 rel err in numpy):
per frame (noise u[256], amp[65]):
    x = 2u - 1
    a = x @ A        # [768]  packed spectrum slots (Re | Im' | Re+Im')
    b = amp @ B      # [768]
    m = a * b        # elementwise (Karatsuba 3-mult complex product)
    out = m @ E      # [256]  inverse transform, all +/- folded into E

A [256,768], B [65,768], E [768,256] are fixed host-precomputed matrices
(DFT of the zero-padded noise, amp->windowed-impulse->DFT, and the
second-half irfft combined with the Karatsuba recombination).

Data parallel over 8 cores: 65536 frames -> 8192 frames/core.
"""

import os
import sys

import numpy as np

os.environ.setdefault("MYCRO_LOCAL_CACHE", "1")

HOP = 256
NB = 65
B_DIM = 16
F_DIM = 4096
NCORES = 8
FRAMES = B_DIM * F_DIM
FR_PER_CORE = FRAMES // NCORES  # 8192
BLK = 512                        # frames per block
MM_DT = "float16"  # matmul-input dtype: float16 (1 cyc/row) or float32 (4 cyc/row)


# ---------------------------------------------------------------- matrices
def _build_matrices():
    FS = 128
    NFFT = 512
    t = np.arange(HOP)
    m = np.arange(257)
    W = np.exp(-2j * np.pi * np.outer(t, m) / NFFT)  # [256, 257]

    eye = np.eye(NB)
    ir = np.fft.irfft(eye, axis=-1)  # [65, 128]
    ir = np.roll(ir, FS // 2, axis=-1)
    n = np.arange(FS)
    win = 0.5 * (1.0 - np.cos(2.0 * np.pi * n / FS))
    ir = ir * win
    ir = np.pad(ir, ((0, 0), (0, HOP - FS)))
    M_imp = np.roll(ir, -(FS // 2), axis=-1)  # [65, 256]

    D = (M_imp @ W) * ((-1.0) ** m)[None, :]  # [65, 257]

    U = np.zeros((257, HOP))
    V = np.zeros((257, HOP))
    for mm in range(257):
        spec = np.zeros(257, complex); spec[mm] = 1.0
        U[mm] = np.fft.irfft(spec, n=NFFT)[HOP:]
        spec = np.zeros(257, complex); spec[mm] = 1j
        V[mm] = np.fft.irfft(spec, n=NFFT)[HOP:]

    A1 = W[:, 0:256].real.copy()
    A2 = np.empty((HOP, 256))
    A2[:, 0] = W[:, 256].real
    A2[:, 1:] = W[:, 1:256].imag
    A3 = A1 + A2

    B1 = D[:, 0:256].real.copy()
    B2 = np.empty((NB, 256))
    B2[:, 0] = D[:, 256].real
    B2[:, 1:] = D[:, 1:256].imag
    B3 = B1 + B2

    E1 = np.empty((256, HOP)); E2 = np.empty((256, HOP)); E3 = np.empty((256, HOP))
    E1[0] = U[0]; E2[0] = U[256]; E3[0] = 0.0
    E1[1:] = U[1:256] - V[1:256]
    E2[1:] = -U[1:256] - V[1:256]
    E3[1:] = V[1:256]

    ndt = np.float16 if MM_DT == "float16" else np.float32
    A = np.ascontiguousarray(np.concatenate([A1, A2, A3], axis=1), ndt)
    Bm = np.ascontiguousarray(np.concatenate([B1, B2, B3], axis=1), ndt)
    E = np.ascontiguousarray(np.concatenate([E1, E2, E3], axis=0), ndt)
    return A, Bm, E


# ---------------------------------------------------------------- bass kernel
def _emit_kernel(ctx, tc, noise, amp, a_cst, b_cst, e_cst, out, n_frames):
    import concourse.bass as bass
    import concourse.mybir as mybir
    from concourse.masks import make_identity

    nc = tc.nc
    f32 = mybir.dt.float32
    f32r = mybir.dt.float32r
    Copy = mybir.ActivationFunctionType.Copy

    mdt = getattr(mybir.dt, MM_DT)

    nblk = n_frames // BLK
    SUB = BLK // 128  # 4 frame-subtiles per block

    singles = ctx.enter_context(tc.tile_pool(name="singles", bufs=1))
    p_in = ctx.enter_context(tc.tile_pool(name="p_in", bufs=3))
    p_xt = ctx.enter_context(tc.tile_pool(name="p_xt", bufs=2))
    p_mid = ctx.enter_context(tc.tile_pool(name="p_mid", bufs=2))
    p_out = ctx.enter_context(tc.tile_pool(name="p_out", bufs=3))
    ps_t = ctx.enter_context(tc.tile_pool(name="ps_t", bufs=2, space="PSUM"))
    ps_a = ctx.enter_context(tc.tile_pool(name="ps_a", bufs=2, space="PSUM"))
    ps_b = ctx.enter_context(tc.tile_pool(name="ps_b", bufs=2, space="PSUM"))
    ps_o = ctx.enter_context(tc.tile_pool(name="ps_o", bufs=2, space="PSUM"))

    ident = singles.tile([128, 128], f32)
    make_identity(nc, ident)

    a_sb = singles.tile([128, 2, 768], mdt)
    nc.sync.dma_start(out=a_sb, in_=a_cst.rearrange("(kt p) s -> p kt s", p=128))
    b_sb = singles.tile([NB, 768], mdt)
    nc.sync.dma_start(out=b_sb, in_=b_cst)
    e_sb = singles.tile([128, 6, 256], mdt)
    nc.sync.dma_start(out=e_sb, in_=e_cst.rearrange("(c p) t -> p c t", p=128))

    nv = noise.rearrange("(nb fi p) t -> nb fi p t", fi=SUB, p=128)
    av = amp.rearrange("(nb fi p) k -> nb fi p k", fi=SUB, p=128)
    ov = out.rearrange("(nb fi p) t -> nb fi p t", fi=SUB, p=128)

    for b in range(nblk):
        # ---- load
        u_t = p_in.tile([128, SUB, HOP], f32, tag="u")
        nc.sync.dma_start(out=u_t, in_=nv[b].rearrange("fi p t -> p fi t"))
        amp_t = p_in.tile([128, SUB, NB], f32, tag="amp")
        nc.sync.dma_start(out=amp_t, in_=av[b].rearrange("fi p k -> p fi k"))

        # ---- transpose noise to [t, f] (and fuse x = 2u - 1 into the copy)
        xt = []
        for h in range(2):
            pt = ps_t.tile([128, BLK], f32, tag="pt")
            for fi in range(SUB):
                nc.tensor.transpose(
                    pt[:, fi * 128:(fi + 1) * 128],
                    u_t[:, fi, h * 128:(h + 1) * 128],
                    ident,
                )
            x_h = p_xt.tile([128, BLK], mdt, tag=f"xt{h}")
            nc.scalar.activation(out=x_h, in_=pt, func=Copy, bias=-1.0, scale=2.0)
            xt.append(x_h)

        # ---- transpose amp to [k, f]
        pt = ps_t.tile([128, BLK], f32, tag="pt")
        for fi in range(SUB):
            nc.tensor.transpose(
                pt[:NB, fi * 128:(fi + 1) * 128],
                amp_t[:, fi, :],
                ident,
            )
        amp_T = p_xt.tile([NB, BLK], mdt, tag="ampT")
        nc.scalar.activation(out=amp_T, in_=pt[:NB, :], func=Copy)

        # ---- b = amp @ B ; a = x @ A ; m = a * b     (6 slot-chunks of 128)
        m_sb = p_mid.tile([128, 6, BLK], mdt, tag="m")
        bc_sb = p_mid.tile([128, 6, BLK], mdt, tag="bc")
        for c in range(6):
            pb = ps_b.tile([128, BLK], f32, tag="pb")
            nc.tensor.matmul(
                pb,
                b_sb[:, c * 128:(c + 1) * 128],
                amp_T,
                start=True, stop=True,
            )
            nc.scalar.activation(out=bc_sb[:, c, :], in_=pb, func=Copy)

            pa = ps_a.tile([128, BLK], f32, tag="pa")
            for k in range(2):
                nc.tensor.matmul(
                    pa,
                    a_sb[:, k, c * 128:(c + 1) * 128],
                    xt[k],
                    start=(k == 0), stop=(k == 1),
                )
            nc.vector.tensor_mul(m_sb[:, c, :], pa, bc_sb[:, c, :])

        # ---- out[f, :] = sum_c m[c-slots, f] @ E[c]   (m-chunk is stationary)
        o_sb = p_out.tile([128, SUB, HOP], f32, tag="o")
        for fi in range(SUB):
            po = ps_o.tile([128, HOP], f32, tag="po")
            for c in range(6):
                nc.tensor.matmul(
                    po,
                    m_sb[:, c, fi * 128:(fi + 1) * 128],
                    e_sb[:, c, :],
                    start=(c == 0), stop=(c == 5),
                )
            nc.scalar.activation(out=o_sb[:, fi, :], in_=po, func=Copy)

        nc.sync.dma_start(out=ov[b].rearrange("fi p t -> p fi t"), in_=o_sb)


def build_nc(n_frames=FR_PER_CORE):
    import concourse.bacc as bacc
    import concourse.mybir as mybir
    import concourse.tile as tile

    f32 = mybir.dt.float32
    mdt = getattr(mybir.dt, MM_DT)
    nc = bacc.Bacc("TRN2", target_bir_lowering=False, debug=False)
    noise = nc.dram_tensor("noise", [n_frames, HOP], f32, kind="ExternalInput").ap()
    amp = nc.dram_tensor("amp", [n_frames, NB], f32, kind="ExternalInput").ap()
    a_cst = nc.dram_tensor("a_cst", [HOP, 768], mdt, kind="ExternalInput").ap()
    b_cst = nc.dram_tensor("b_cst", [NB, 768], mdt, kind="ExternalInput").ap()
    e_cst = nc.dram_tensor("e_cst", [768, HOP], mdt, kind="ExternalInput").ap()
    out = nc.dram_tensor("out", [n_frames, HOP], f32, kind="ExternalOutput").ap()

    from contextlib import ExitStack

    with tile.TileContext(nc) as tc, ExitStack() as ctx:
        _emit_kernel(ctx, tc, noise, amp, a_cst, b_cst, e_cst, out, n_frames)
    nc.compile()
    return nc


_CACHE = {}


def _get(n_frames=FR_PER_CORE):
    if n_frames not in _CACHE:
        _CACHE[n_frames] = (build_nc(n_frames), _build_matrices())
    return _CACHE[n_frames]


def run_sharded(noise_flat, amp_flat, n_frames_per_core, n_cores, trace=False):
    from concourse import bass_utils

    nc, (A, Bm, E) = _get(n_frames_per_core)
    in_maps = []
    for i in range(n_cores):
        lo, hi = i * n_frames_per_core, (i + 1) * n_frames_per_core
        in_maps.append({
            "noise": np.ascontiguousarray(noise_flat[lo:hi]),
            "amp": np.ascontiguousarray(amp_flat[lo:hi]),
            "a_cst": A, "b_cst": Bm, "e_cst": E,
        })
    res = bass_utils.run_bass_kernel_spmd(
        nc, in_maps, core_ids=list(range(n_cores)), trace=trace
    )
    out = np.concatenate([res.results[i]["out"] for i in range(n_cores)], axis=0)
    return out, res


def kernel(filter_bank, noise_u):
    fb = np.asarray(filter_bank, np.float32).reshape(-1, NB)
    nu = np.asarray(noise_u, np.float32).reshape(-1, HOP)
    out, _ = run_sharded(nu, fb, FR_PER_CORE, NCORES)
    return out.reshape(B_DIM, F_DIM * HOP, 1).astype(np.float32)


if __name__ == "__main__":
    nc = build_nc(BLK)
    print("built OK")


# revision 13
# speedup vs baseline: 21922.4809x; 21922.4809x over previous
"""Trainium2 Bass kernel for nn_NoiseFilter.

Math (derived from the reference, validated to ~6e-7 rel err in numpy):
per frame (noise u[256], amp[65]):
    x = 2u - 1
    a = x @ A        # [768]  packed spectrum slots (Re | Im' | Re+Im')
    b = amp @ B      # [768]
    m = a * b        # elementwise (Karatsuba 3-mult complex product)
    out = m @ E      # [256]  inverse transform, all +/- folded into E

A [256,768], B [65,768], E [768,256] are fixed host-precomputed matrices
(DFT of the zero-padded noise, amp->windowed-impulse->DFT, and the
second-half irfft combined with the Karatsuba recombination).

On-chip dataflow (per 512-frame block, frames sharded 8 ways):
    u16   = cast-DMA(noise block)          # SWDGE fp32->fp16
    xT    = xbar-transpose-DMA(u16)        # [t, f] fp16, 8x [128,128]
    ampT  = PE-transpose(cast-DMA(amp))    # [k, f] fp16
    a     = A'(t-tiles) @ xT               # PE, fp16 in / fp32 PSUM, A' = 2A
    b     = B @ ampT                       # PE -> ACT copy to SBUF fp16
    m     = (a + cbias) * b                # custom DVE op, fp16 out
    out   = sum_c m_chunk @ E_chunk        # PE (m stationary), fp32 PSUM
    DMA out

Data parallel over 8 cores: 65536 frames -> 8192 frames/core.
"""

import os
import re as _re
import sys

import numpy as np

os.environ.setdefault("MYCRO_LOCAL_CACHE", "1")

HOP = 256
NB = 65
B_DIM = 16
F_DIM = 4096
NCORES = 8
FRAMES = B_DIM * F_DIM
FR_PER_CORE = FRAMES // NCORES  # 8192
BLK = 512                        # frames per block


# ---------------------------------------------------------------- matrices
def _build_matrices():
    FS = 128
    NFFT = 512
    t = np.arange(HOP)
    m = np.arange(257)
    W = np.exp(-2j * np.pi * np.outer(t, m) / NFFT)  # [256, 257]

    eye = np.eye(NB)
    ir = np.fft.irfft(eye, axis=-1)  # [65, 128]
    ir = np.roll(ir, FS // 2, axis=-1)
    n = np.arange(FS)
    win = 0.5 * (1.0 - np.cos(2.0 * np.pi * n / FS))
    ir = ir * win
    ir = np.pad(ir, ((0, 0), (0, HOP - FS)))
    M_imp = np.roll(ir, -(FS // 2), axis=-1)  # [65, 256]

    D = (M_imp @ W) * ((-1.0) ** m)[None, :]  # [65, 257]

    U = np.zeros((257, HOP))
    V = np.zeros((257, HOP))
    for mm in range(257):
        spec = np.zeros(257, complex); spec[mm] = 1.0
        U[mm] = np.fft.irfft(spec, n=NFFT)[HOP:]
        spec = np.zeros(257, complex); spec[mm] = 1j
        V[mm] = np.fft.irfft(spec, n=NFFT)[HOP:]

    A1 = W[:, 0:256].real.copy()
    A2 = np.empty((HOP, 256))
    A2[:, 0] = W[:, 256].real
    A2[:, 1:] = W[:, 1:256].imag
    A3 = A1 + A2

    B1 = D[:, 0:256].real.copy()
    B2 = np.empty((NB, 256))
    B2[:, 0] = D[:, 256].real
    B2[:, 1:] = D[:, 1:256].imag
    B3 = B1 + B2

    E1 = np.empty((256, HOP)); E2 = np.empty((256, HOP)); E3 = np.empty((256, HOP))
    E1[0] = U[0]; E2[0] = U[256]; E3[0] = 0.0
    E1[1:] = U[1:256] - V[1:256]
    E2[1:] = -U[1:256] - V[1:256]
    E3[1:] = V[1:256]

    A = np.concatenate([A1, A2, A3], axis=1)   # [256, 768]
    Bm = np.concatenate([B1, B2, B3], axis=1)  # [65, 768]
    E = np.concatenate([E1, E2, E3], axis=0)   # [768, 256]

    A2x = np.ascontiguousarray(2.0 * A, np.float16)           # folds x = 2u-1 scale
    cbias = np.ascontiguousarray(-A.sum(axis=0), np.float32)  # folds the -1 shift
    Bm = np.ascontiguousarray(Bm, np.float16)
    E = np.ascontiguousarray(E, np.float16)
    return A2x, Bm, cbias, E


# ------------------------------------------------------- custom DVE op
_NF_OP = None


def _get_custom_op():
    """out = (in0 + s0) * in1, s0 a per-partition scalar."""
    global _NF_OP
    if _NF_OP is not None:
        return _NF_OP
    import concourse.dve_ops as dve_ops
    from concourse.dve_ops import OPS, DveOp
    from concourse.dve_spec import C0, Spec, Src0, Src1

    for existing in OPS:
        if existing.name == "NF_ADDMUL":
            _NF_OP = existing
            return _NF_OP

    op = DveOp(
        "NF_ADDMUL",
        Spec(
            body=(Src0 + C0) * Src1,
            reference=lambda in0, in1, s0, s1, imm2: (in0 + s0) * in1,
        ),
        subdim=False,
        uops_sha={},
    )
    OPS.append(op)
    dve_ops.CUSTOM_DVE_SPECS[op.name] = op.spec
    dve_ops._SUB_OPCODE_FOR_NAME[op.name] = (
        dve_ops._CUSTOM_DVE_ROW_BASE + len(OPS) - 1
    )
    for ver in ("v3", "v4"):
        try:
            op.compile(ver)
        except ValueError as e:
            m = _re.search(r'\]="([0-9a-f]+)"', str(e))
            if not m:
                raise
            op.uops_sha[ver] = m.group(1)
            op.compile(ver)
    _NF_OP = op
    return op


# ---------------------------------------------------------------- bass kernel
def _emit_kernel(ctx, tc, noise, amp, a_cst, b_cst, c_cst, e_cst, out, n_frames, reps=1):
    import concourse.mybir as mybir
    from concourse.masks import make_identity

    nc = tc.nc
    f32 = mybir.dt.float32
    f16 = mybir.dt.float16
    Copy = mybir.ActivationFunctionType.Copy
    nf_op = _get_custom_op()

    nblk = n_frames // BLK
    SUB = BLK // 128  # 4 frame-subtiles per block

    singles = ctx.enter_context(tc.tile_pool(name="singles", bufs=1))
    p_in = ctx.enter_context(tc.tile_pool(name="p_in", bufs=3))
    p_xt = ctx.enter_context(tc.tile_pool(name="p_xt", bufs=2))
    p_mid = ctx.enter_context(tc.tile_pool(name="p_mid", bufs=2))
    p_out = ctx.enter_context(tc.tile_pool(name="p_out", bufs=3))
    ps_a = ctx.enter_context(tc.tile_pool(name="ps_a", bufs=3, space="PSUM"))
    ps_b = ctx.enter_context(tc.tile_pool(name="ps_b", bufs=2, space="PSUM"))
    ps_t = ctx.enter_context(tc.tile_pool(name="ps_t", bufs=1, space="PSUM"))
    ps_o = ctx.enter_context(tc.tile_pool(name="ps_o", bufs=2, space="PSUM"))

    ident = singles.tile([128, 128], f16)
    make_identity(nc, ident)

    a_sb = singles.tile([128, 2, 768], f16)
    nc.sync.dma_start(out=a_sb, in_=a_cst.rearrange("(kt p) s -> p kt s", p=128))
    b_sb = singles.tile([NB, 768], f16)
    nc.sync.dma_start(out=b_sb, in_=b_cst)
    e_sb = singles.tile([128, 6, 256], f16)
    nc.sync.dma_start(out=e_sb, in_=e_cst.rearrange("(c p) t -> p c t", p=128))
    cb_sb = singles.tile([128, 6], f32)
    nc.sync.dma_start(out=cb_sb, in_=c_cst.rearrange("(c p) -> p c", p=128))

    nv = noise.rearrange("(nb fi p) t -> nb fi p t", fi=SUB, p=128)
    av = amp.rearrange("(nb fi p) k -> nb fi p k", fi=SUB, p=128)
    ov = out.rearrange("(nb fi p) t -> nb fi p t", fi=SUB, p=128)

    for rep in range(reps):
      for b in range(nblk):
        # ---- load (SWDGE cast fp32 -> fp16)
        u16 = p_in.tile([128, SUB, HOP], f16, tag="u")
        nc.gpsimd.dma_start(out=u16, in_=nv[b].rearrange("fi p t -> p fi t"))
        amp16 = p_in.tile([128, SUB, NB], f16, tag="amp")
        nc.gpsimd.dma_start(out=amp16, in_=av[b].rearrange("fi p k -> p fi k"))

        # ---- noise transpose to [t, f] via PE (fp16, 1 cyc/row)
        xt = []
        for h in range(2):
            ptx = ps_t.tile([128, BLK], f16, tag="pt")
            for fi in range(SUB):
                nc.tensor.transpose(
                    ptx[:, fi * 128:(fi + 1) * 128],
                    u16[:, fi, h * 128:(h + 1) * 128],
                    ident,
                )
            x_h = p_xt.tile([128, BLK], f16, tag=f"xt{h}")
            nc.scalar.activation(out=x_h, in_=ptx, func=Copy)
            xt.append(x_h)

        # ---- amp transpose to [k, f] via PE
        pt = ps_t.tile([128, BLK], f16, tag="pt")
        for fi in range(SUB):
            nc.tensor.transpose(
                pt[:NB, fi * 128:(fi + 1) * 128],
                amp16[:, fi, :],
                ident,
            )
        amp_T = p_xt.tile([NB, BLK], f16, tag="ampT")
        nc.vector.tensor_copy(amp_T, pt[:NB, :])

        # ---- b = amp @ B (chunk pairs share one PSUM tile + one ACT copy)
        bc_sb = p_mid.tile([128, 6, BLK], f16, tag="bc")
        for c in range(6):
            pb = ps_b.tile([128, BLK], f32, tag="pb")
            nc.tensor.matmul(
                pb,
                b_sb[:, c * 128:(c + 1) * 128],
                amp_T,
                start=True, stop=True,
            )
            nc.scalar.activation(out=bc_sb[:, c, :], in_=pb, func=Copy)

        # ---- a = x @ A' ; m = (a + cbias) * b
        m_sb = p_mid.tile([128, 6, BLK], f16, tag="m")
        for c in range(6):
            pa = ps_a.tile([128, BLK], f32, tag="pa")
            for k in range(2):
                nc.tensor.matmul(
                    pa,
                    a_sb[:, k, c * 128:(c + 1) * 128],
                    xt[k],
                    start=(k == 0), stop=(k == 1),
                )
            nc.vector._custom_dve(
                nf_op,
                out=m_sb[:, c, :],
                in0=pa,
                in1=bc_sb[:, c, :],
                s0=cb_sb[:, c:c + 1],
            )

        # ---- out[f, :] = sum_c m[c-slots, f] @ E[c]   (m-chunk is stationary)
        o_sb = p_out.tile([128, SUB, HOP], f32, tag="o")
        for fp in range(SUB // 2):
            po = ps_o.tile([128, 2, HOP], f32, tag="po")
            for j in range(2):
                fi = 2 * fp + j
                for c in range(6):
                    nc.tensor.matmul(
                        po[:, j, :],
                        m_sb[:, c, fi * 128:(fi + 1) * 128],
                        e_sb[:, c, :],
                        start=(c == 0), stop=(c == 5),
                    )
            nc.scalar.activation(
                out=o_sb[:, 2 * fp:2 * fp + 2, :], in_=po, func=Copy)

        nc.sync.dma_start(out=ov[b].rearrange("fi p t -> p fi t"), in_=o_sb)


def build_nc(n_frames=FR_PER_CORE, reps=1):
    import concourse.bacc as bacc
    import concourse.mybir as mybir
    import concourse.tile as tile

    f32 = mybir.dt.float32
    f16 = mybir.dt.float16
    nc = bacc.Bacc("TRN2", target_bir_lowering=False, debug=False)
    noise = nc.dram_tensor("noise", [n_frames, HOP], f32, kind="ExternalInput").ap()
    amp = nc.dram_tensor("amp", [n_frames, NB], f32, kind="ExternalInput").ap()
    a_cst = nc.dram_tensor("a_cst", [HOP, 768], f16, kind="ExternalInput").ap()
    b_cst = nc.dram_tensor("b_cst", [NB, 768], f16, kind="ExternalInput").ap()
    c_cst = nc.dram_tensor("c_cst", [768], f32, kind="ExternalInput").ap()
    e_cst = nc.dram_tensor("e_cst", [768, HOP], f16, kind="ExternalInput").ap()
    out = nc.dram_tensor("out", [n_frames, HOP], f32, kind="ExternalOutput").ap()

    from contextlib import ExitStack

    with tile.TileContext(nc) as tc, ExitStack() as ctx:
        _emit_kernel(ctx, tc, noise, amp, a_cst, b_cst, c_cst, e_cst, out,
                     n_frames, reps=reps)
    nc.compile()
    return nc


_CACHE = {}


def _get(n_frames=FR_PER_CORE, reps=1):
    key = (n_frames, reps)
    if key not in _CACHE:
        _CACHE[key] = (build_nc(n_frames, reps=reps), _build_matrices())
    return _CACHE[key]


def run_sharded(noise_flat, amp_flat, n_frames_per_core, n_cores, trace=False):
    from concourse import bass_utils

    nc, (A, Bm, cb, E) = _get(n_frames_per_core)
    in_maps = []
    for i in range(n_cores):
        lo, hi = i * n_frames_per_core, (i + 1) * n_frames_per_core
        in_maps.append({
            "noise": np.ascontiguousarray(noise_flat[lo:hi]),
            "amp": np.ascontiguousarray(amp_flat[lo:hi]),
            "a_cst": A, "b_cst": Bm, "c_cst": cb, "e_cst": E,
        })
    res = bass_utils.run_bass_kernel_spmd(
        nc, in_maps, core_ids=list(range(n_cores)), trace=trace
    )
    out = np.concatenate([res.results[i]["out"] for i in range(n_cores)], axis=0)
    return out, res


def kernel(filter_bank, noise_u):
    fb = np.asarray(filter_bank, np.float32).reshape(-1, NB)
    nu = np.asarray(noise_u, np.float32).reshape(-1, HOP)
    out, _ = run_sharded(nu, fb, FR_PER_CORE, NCORES)
    return out.reshape(B_DIM, F_DIM * HOP, 1).astype(np.float32)


if __name__ == "__main__":
    nc = build_nc(BLK)
    print("built OK")
